# revision 7
# baseline (speedup 1.0000x reference)
"""Capsule-routing kernel for 8 Trainium2 NeuronCores (Bass/Tile).

Problem: x [64,2048,16] f32, W [32,2048,16,16] f32 -> v [64,32,16] f32
  u_hat[b,n,j,d] = sum_i W[n,j,d,i] * x[b,j,i]
  3 rounds of dynamic routing (softmax over n, weighted sums over j, squash).

Sharding: J (input capsules, 2048) split 8 ways -> 256 j per core. All
routing state except s is local to a core; s = sum_j c*u_hat [64,32,16]
(128 KiB) is AllReduced across the 8 cores each round (softmax over n is
elementwise in j, so it stays local). Inputs are pre-cast to bf16 and
pre-transposed on the host into DMA-friendly layouts.

Per-core device plan:
  * u_hat built once on the TensorEngine: per block of 8 j's the 8 small
    [16x16] W matrices are placed on the diagonal of a [128,128] moving
    tile (zeros memset once; 8 strided DMAs refill only the diagonal
    blocks), multiplied against a stationary x tile [(j8,i)=128, b=64].
    Output lands in PSUM as [b(+64*par), (n, j8, d)], evacuated to SBUF
    as bf16 u_hat U[(par,b)=128, n, d, jq=128].
  * s-pass: fused scalar_tensor_tensor (mult + per-partition total-reduce
    via accum_out), one instr per (n,d); partial s folded over the j-parity
    partition halves, AllReduced via DRAM, then squash (sqrt via
    exp(0.5*ln(x)) to stay in one ACT table set with softmax's exp).
  * t-pass (logit update): per (n,d) scalar_tensor_tensor chains with the
    per-partition scalar v[b,n,d], accumulating in two bf16 chains of
    depth 8, then summed into f32 logits.
"""

import numpy as np

# ---------------------------------------------------------------- constants
B, N, D, I, J = 64, 32, 16, 16, 2048
CORES = 8
JS = J // CORES          # 256 local j per core
JBLK = JS // 8           # 32 blocks of 8 j
PAIRS = JBLK // 2        # 16 block-pairs
Q = JS // 2              # 128 j per parity group
ND = N * D               # 512
ROUNDS = 3
EPS = 1e-7

_CACHE = {}


def _patch_tile_drain():
    """walrus in this container rejects instructions with >2 sem waits; the
    Tile tail drain waits on every logical proc at once.  Replace it with a
    chain of single-wait absorber nops (same SP engine, program order) so the
    final drain needs no waits."""
    import concourse.tile as tile
    from concourse.vector_clock import ScopedClock, VectorClock

    if getattr(tile.TileContext, "_caps_drain_patched", False):
        return

    def _drain_and_barrier(self, tick_clock, wait_clock):
        gc = tick_clock.global_clock
        n = len(gc)
        for proc in range(n):
            t = gc[proc]
            if t > 0:
                vc = VectorClock([0] * n)
                vc.require_at_least(proc, t)
                nop = self.nc.sync.nop()
                wait_clock.add_sem_waits(nop.ins, ScopedClock({None: vc}))
        self.nc.sync.drain()
        self.nc.all_engine_barrier()
        popped = self.nc._tile_sem_poison_stack.pop()
        assert popped is self._sem_poison
        self.nc.clear_and_free_semaphores(list(self.sems.allocated().values()))
        self.nc.all_engine_barrier()

    tile.TileContext._drain_and_barrier = _drain_and_barrier
    tile.TileContext._caps_drain_patched = True


def _split_excess_waits(nc, mybir, limit=2):
    """walrus codegen in this container accepts at most `limit` sem waits per
    instruction. Hoist excess waits onto same-engine NOPs inserted right
    before the offending instruction (engine streams execute block order, so
    the waits still complete before it runs)."""
    idx = 0
    for fn in nc.m.functions:
        for bb in fn.blocks:
            out = []
            changed = False
            for ins in bb.instructions:
                si = ins.sync_info
                if si is not None and si.on_wait and len(si.on_wait) > limit:
                    waits = list(si.on_wait)
                    keep, rest = waits[-limit:], waits[:-limit]
                    for k in range(0, len(rest), limit):
                        nop = mybir.InstNoOp(
                            name=f"I-wsplit-{idx}", ins=[], outs=[])
                        idx += 1
                        nop.engine = ins.engine
                        nop.sync_info = mybir.SyncInfo(
                            on_wait=rest[k:k + limit], on_update=[])
                        out.append(nop)
                    ins.sync_info = mybir.SyncInfo(
                        on_wait=keep, on_update=list(si.on_update))
                    changed = True
                out.append(ins)
            if changed:
                bb.instructions = out


def _build_program():
    import concourse.bass as bass
    import concourse.tile as tile
    from concourse import mybir

    _patch_tile_drain()

    bf16 = mybir.dt.bfloat16
    f32 = mybir.dt.float32
    ALU = mybir.AluOpType
    ACT = mybir.ActivationFunctionType

    nc = bass.Bass("TRN2", target_bir_lowering=False, debug=False,
                   num_devices=CORES)
    eps_t = nc.alloc_sbuf_tensor("const-float32-eps", [128, 1], f32)
    nc.gpsimd.memset(eps_t.ap(), EPS)
    nc.const_aps.aps[(f32, EPS)] = eps_t.ap()
    x_t = nc.dram_tensor("x_t", [128, JBLK, B], bf16, kind="ExternalInput")
    w_h = nc.dram_tensor("w_h", [JBLK, 8, I, N, D], bf16,
                         kind="ExternalInput")
    v_out = nc.dram_tensor("v_out", [B, ND], f32, kind="ExternalOutput")

    groups = [list(range(CORES))]

    with tile.TileContext(nc) as tc:
        with tc.tile_pool(name="persist", bufs=1) as per:
            U = per.tile([128, N, D, Q], bf16, tag="u")       # 128 KiB/p
            L = per.tile([128, N, Q], f32, tag="logits")      # 16 KiB/p
            V = per.tile([128, N, D], f32, tag="v")           # 2 KiB/p
            SP_ = per.tile([128, ND], f32, tag="spart")       # 2 KiB/p
            SFULL = per.tile([128, N, D], f32, tag="sfull")   # 2 KiB/p

            # ------------------------------------------------ u_hat build
            with tc.tile_pool(name="xt", bufs=2) as xt_pool, \
                 tc.tile_pool(name="wbd", bufs=2) as wbd_pool, \
                 tc.tile_pool(name="bps", bufs=2, space="PSUM") as ps_pool:
                for pair in range(PAIRS):
                    xt = xt_pool.tile([128, 2, B], bf16, tag="xt")
                    nc.sync.dma_start(
                        xt[:], x_t[:, 2 * pair:2 * pair + 2, :])
                    wtiles = []
                    for h in range(2):
                        jb = 2 * pair + h
                        w = wbd_pool.tile([128, N, 128], bf16, tag="wbd")
                        if pair == 0:
                            nc.gpsimd.memset(w[:], 0.0)
                        for j8 in range(8):
                            nc.sync.dma_start(
                                w[j8 * 16:(j8 + 1) * 16, :,
                                  j8 * 16:(j8 + 1) * 16],
                                w_h[jb, j8])
                        wtiles.append(w)
                    pss = [ps_pool.tile([128, 16, 8, D], f32, tag="ps",
                                        name=f"ps_{pair}_{g}")
                           for g in range(2)]
                    for h in range(2):
                        for ng in range(8):
                            ps = pss[ng // 4]
                            nc.tensor.matmul(
                                ps[h * 64:(h + 1) * 64,
                                   (ng % 4) * 4:(ng % 4) * 4 + 4, :, :],
                                lhsT=xt[:, h, :],
                                rhs=wtiles[h][:, ng * 4:(ng + 1) * 4, :],
                                start=True, stop=True)
                    for g in range(2):
                        # U slice [(p), n(16), d, j8(8)] -> dims (n, j8, d)
                        dst = U[:, g * 16:(g + 1) * 16, :,
                                pair * 8:(pair + 1) * 8]
                        dst = dst.transpose([0, 1, 3, 2])
                        if g == 0:
                            nc.vector.tensor_copy(dst, pss[g][:])
                        else:
                            nc.scalar.copy(dst, pss[g][:])

            # ------------------------------------------------ routing
            with tc.tile_pool(name="route", bufs=1) as rp, \
                 tc.tile_pool(name="tchain", bufs=2) as tp, \
                 tc.tile_pool(name="ccd", bufs=6, space="DRAM") as ccd:
                C = rp.tile([128, N, Q], bf16, tag="c")       # exp/c
                JUNK = rp.tile([128, Q], bf16, tag="junk")
                SQ = rp.tile([128, N, D], f32, tag="sq")
                S2 = rp.tile([128, N], f32, tag="s2")
                LNS = rp.tile([128, N], f32, tag="lns")
                SQR = rp.tile([128, N], f32, tag="sqr")
                DEN = rp.tile([128, N], f32, tag="den")
                RCD = rp.tile([128, N], f32, tag="rcd")
                SCL = rp.tile([128, N], f32, tag="scl")
                SPB = rp.tile([128, ND], f32, tag="spb")
                RCP = rp.tile([128, Q], f32, tag="rcp")

                for it in range(ROUNDS):
                    # ---------------- c = softmax(L) over n (it>0)
                    if it > 0:
                        nc.scalar.activation(C[:], L[:], ACT.Exp)
                        tre = tp.tile([128, 16, Q], f32, tag="tchain")
                        nc.vector.tensor_add(
                            tre[:], C[:, 0:16, :], C[:, 16:32, :])
                        for lv in (8, 4, 2, 1):
                            nc.vector.tensor_add(
                                tre[:, 0:lv, :], tre[:, 0:lv, :],
                                tre[:, lv:2 * lv, :])
                        nc.vector.reciprocal(RCP[:], tre[:, 0, :])
                        nc.vector.tensor_mul(
                            C[:], C[:],
                            RCP[:].unsqueeze(1).broadcast_to([128, N, Q]))

                    # ---------------- s partials: fused mult+reduce
                    for n in range(N):
                        for d in range(D):
                            acc = SP_[:, n * D + d:n * D + d + 1]
                            if it == 0:
                                nc.vector.scalar_tensor_tensor(
                                    JUNK[:], U[:, n, d, :], 1.0 / N,
                                    U[:, n, d, :],
                                    ALU.mult, ALU.bypass, accum_out=acc)
                            else:
                                nc.vector.scalar_tensor_tensor(
                                    JUNK[:], U[:, n, d, :], 0.0,
                                    C[:, n, :],
                                    ALU.bypass, ALU.mult, accum_out=acc)

                    # fold j-parity halves, AllReduce across cores
                    nc.sync.dma_start(SPB[0:64, :], SP_[64:128, :])
                    nc.vector.tensor_add(
                        SFULL[0:64, :, :].rearrange("p n d -> p (n d)"),
                        SP_[0:64, :], SPB[0:64, :])
                    cc_in = ccd.tile([B, ND], f32, tag="ccin")
                    cc_out = ccd.tile([B, ND], f32, tag="ccout",
                                      addr_space="Shared")
                    nc.sync.dma_start(
                        cc_in[:],
                        SFULL[0:64, :, :].rearrange("p n d -> p (n d)"))
                    nc.gpsimd.collective_compute(
                        "AllReduce", ALU.add, replica_groups=groups,
                        ins=[cc_in[:]], outs=[cc_out[:]])
                    nc.sync.dma_start(
                        SFULL[0:64, :, :].rearrange("p n d -> p (n d)"),
                        cc_out[:])

                    # ---------------- v = squash(s)   (rows 0:64 only)
                    nc.scalar.activation(SQ[0:64], SFULL[0:64], ACT.Square)
                    nc.vector.tensor_reduce(
                        S2[0:64], SQ[0:64], mybir.AxisListType.X, ALU.add)
                    nc.scalar.activation(LNS[0:64], S2[0:64], ACT.Ln,
                                         bias=EPS)
                    nc.scalar.activation(SQR[0:64], LNS[0:64], ACT.Exp,
                                         scale=0.5)
                    nc.vector.tensor_scalar_add(DEN[0:64], S2[0:64], 1.0)
                    nc.vector.reciprocal(RCD[0:64], DEN[0:64])
                    nc.vector.tensor_mul(SCL[0:64], SQR[0:64], RCD[0:64])
                    nc.vector.tensor_mul(
                        V[0:64], SFULL[0:64],
                        SCL[0:64].unsqueeze(2).broadcast_to([64, N, D]))

                    if it == ROUNDS - 1:
                        nc.sync.dma_start(
                            v_out[:],
                            V[0:64, :, :].rearrange("p n d -> p (n d)"))
                        break

                    # replicate v to the upper partition half
                    nc.sync.dma_start(V[64:128], V[0:64])

                    # ---------------- t-pass: logits += sum_d v*u_hat
                    T0 = tp.tile([128, N, Q], bf16, tag="tchain")
                    T1 = tp.tile([128, N, Q], bf16, tag="tchain")
                    for n in range(N):
                        for d in range(D):
                            Tx = T0 if d < 8 else T1
                            op1 = ALU.bypass if d % 8 == 0 else ALU.add
                            nc.vector.scalar_tensor_tensor(
                                Tx[:, n, :], U[:, n, d, :],
                                V[:, n, d:d + 1], Tx[:, n, :],
                                ALU.mult, op1)
                    if it == 0:
                        nc.vector.tensor_add(L[:], T0[:], T1[:])
                    else:
                        nc.vector.tensor_add(L[:], L[:], T0[:])
                        nc.vector.tensor_add(L[:], L[:], T1[:])

    _split_excess_waits(nc, mybir, limit=1)
    return nc


def _prep_inputs(x, W):
    import ml_dtypes
    bf = ml_dtypes.bfloat16
    in_maps = []
    for c in range(CORES):
        xc = x[:, c * JS:(c + 1) * JS, :]
        xt = (xc.reshape(B, JBLK, 8, I).transpose(2, 3, 1, 0)
              .reshape(128, JBLK, B).astype(bf))
        Wc = W[:, c * JS:(c + 1) * JS, :, :]
        wh = (Wc.reshape(N, JBLK, 8, D, I).transpose(1, 2, 4, 0, 3)
              .astype(bf))
        in_maps.append({"x_t": np.ascontiguousarray(xt),
                        "w_h": np.ascontiguousarray(wh)})
    return in_maps


def _run(in_maps, trace=False):
    from concourse.bass_utils import run_bass_kernel_spmd
    if "nc" not in _CACHE:
        _CACHE["nc"] = _build_program()
    return run_bass_kernel_spmd(_CACHE["nc"], in_maps, list(range(CORES)),
                                trace=trace)


def kernel(x, W, _trace=False):
    x = np.asarray(x, dtype=np.float32)
    W = np.asarray(W, dtype=np.float32)
    res = _run(_prep_inputs(x, W), trace=_trace)
    v = np.asarray(res.results[0]["v_out"], dtype=np.float32)
    out = v.reshape(B, N, D)
    if _trace:
        return out, res
    return out
